# revision 19
# baseline (speedup 1.0000x reference)
"""Trainium2 Bass kernel for AdaptiveSimilarityLearning (pairwise MLP gate).

Computes, for B=512 image/text embeddings (D=512):
  img_h = img @ W1a.T ; txt_h = txt @ W1b.T
  adaptive_sim = sigmoid(sum_d relu(img_h[i,d]+txt_h[j,d]+b1[d]) * W2[d] + b2)
  adaptive_temp = 0.01 + 0.2*sigmoid(sum_d relu(img_t+txt_t+bt1)*Wt2 + bt2)
  out = (0.7 * img@txt.T + 0.3*adaptive_sim) / adaptive_temp

Sharding: rows of the B^2 grid split over 8 NeuronCores (64 rows each);
txt-side tensors and weights replicated. No collectives.

Per-core mapping:
  - d lives on SBUF partitions: txt_hT [d, j] tiles of [128, 512] (bf16).
  - relu(txt_hT[d,j] + img_hT[d,i]) is a per-partition-scalar op: DVE
    tensor_scalar(add, max 0) or ACT activation(Relu, bias) per (i, d-chunk).
  - The weighted partition-reduction sum_d h[d,j]*W2[d] runs on the PE as a
    bf16 matmul with a "delta" stationary S[k, 32v+m] = W2c[k]*(m==v).
  - Row i maps to PE column strip q=i%4 (tile_position=(0,32q)) at delta
    position v=i//4, so 4 consecutive rows' matmuls run on 4 independent
    32-column sub-array strips CONCURRENTLY (~54ns/MM effective) and the
    per-MM LDWEIGHTS is hidden. PSUM accumulator [128, 512] holds output
    row i at partition 32*(i%4) + i//4; cosine is computed in the same
    permuted layout via a host-permuted imgT, and the output DMA
    un-permutes rows on the way to DRAM.

All matmul operands are bf16 (fp32 matmuls lower to 2 half-rate passes on
TRN2); PSUM accumulation and the epilogue stay f32.
"""

import numpy as np

B = 512
D = 512
DH = 256
NCORES = 8
ROWS = B // NCORES  # 64 rows of the pairwise grid per core
KC = D // 128       # 4 contraction chunks
DC_SIM = D // 128   # 4 d-chunks (sim path)
DC_TMP = DH // 128  # 2 d-chunks (temp path)
NQ = 4              # PE column strips
NV = ROWS // NQ     # delta positions used per strip (16)

# measured per-[128,512]-tile costs (ns) used for static load balancing
_COST_DVE = 340.0
_COST_ACT = 704.0

_CACHE = {}


def _build():
    import concourse.tile as tile
    from concourse import bacc, mybir

    f32 = mybir.dt.float32
    bf16 = mybir.dt.bfloat16
    Alu = mybir.AluOpType
    Act = mybir.ActivationFunctionType

    nc = bacc.Bacc("TRN2", target_bir_lowering=False, debug=False,
                   num_devices=NCORES)

    dp = lambda name, shape: nc.dram_tensor(name, shape, bf16,
                                            kind="ExternalInput").ap()
    imgT = dp("imgT", [D, ROWS])        # k x i (natural row order)
    imgTp = dp("imgTp", [D, 128])       # k x permuted+padded strip layout
    txtT = dp("txtT", [D, B])
    w1aT = dp("w1aT", [D, D])
    w1bT = dp("w1bT", [D, D])
    wt1aT = dp("wt1aT", [D, DH])
    wt1bT = dp("wt1bT", [D, DH])
    b1c = dp("b1c", [128, DC_SIM])
    bt1c = dp("bt1c", [128, DC_TMP])
    s_sim = dp("s_sim", [DC_SIM, 128, 32 * NV])
    s_tmp = dp("s_tmp", [DC_TMP, 128, 32 * NV])
    bias2 = nc.dram_tensor("bias2", [128, 2], f32, kind="ExternalInput").ap()
    out_d = nc.dram_tensor("out", [ROWS, B], f32, kind="ExternalOutput").ap()

    with tile.TileContext(nc) as tc:
        with (
            tc.tile_pool(name="consts", bufs=1) as cp,
            tc.tile_pool(name="hpool", bufs=16) as hp,
            tc.tile_pool(name="psacc", bufs=1, space="PSUM") as pacc,
            tc.tile_pool(name="pspre", bufs=4, space="PSUM") as ppre,
            tc.tile_pool(name="epi", bufs=1) as ep,
        ):
            # ---- DMA inputs to SBUF -------------------------------------
            dma_rr = [0]

            def dma_in(dst, srcap):
                eng = nc.sync if dma_rr[0] % 2 == 0 else nc.gpsimd
                dma_rr[0] += 1
                eng.dma_start(dst, srcap)

            def load_chunks(ap, n_k, width, tag, dt=bf16):
                ts = []
                for k in range(n_k):
                    t = cp.tile([128, width], dt, tag=f"{tag}{k}",
                                name=f"{tag}{k}")
                    dma_in(t[:], ap[k * 128:(k + 1) * 128, :])
                    ts.append(t)
                return ts

            # temp-path tensors first so the temp projections start ASAP
            wt1bT_sb = load_chunks(wt1bT, KC, DH, "wt1bT")
            txtT_sb = load_chunks(txtT, KC, B, "txtT")
            bt1_sb = cp.tile([128, DC_TMP], bf16, tag="bt1")
            dma_in(bt1_sb[:], bt1c[:, :])
            wt1aT_sb = load_chunks(wt1aT, KC, DH, "wt1aT")
            imgT_sb = load_chunks(imgT, KC, ROWS, "imgT")
            s_tmp_sb = []
            for c in range(DC_TMP):
                t = cp.tile([128, 32 * NV], bf16, tag=f"stmp{c}", name=f"stmp{c}")
                dma_in(t[:], s_tmp[c, :, :])
                s_tmp_sb.append(t)
            w1bT_sb = load_chunks(w1bT, KC, D, "w1bT")
            b1_sb = cp.tile([128, DC_SIM], bf16, tag="b1")
            dma_in(b1_sb[:], b1c[:, :])
            w1aT_sb = load_chunks(w1aT, KC, D, "w1aT")
            s_sim_sb = []
            for c in range(DC_SIM):
                t = cp.tile([128, 32 * NV], bf16, tag=f"ssim{c}", name=f"ssim{c}")
                dma_in(t[:], s_sim[c, :, :])
                s_sim_sb.append(t)
            imgTp_sb = load_chunks(imgTp, KC, 128, "imgTp")
            bias2_sb = cp.tile([128, 2], f32, tag="bias2")
            dma_in(bias2_sb[:], bias2[:, :])

            # warm the ACT sigmoid table set during the DMA head
            warm = cp.tile([1, 1], f32, tag="warm")
            nc.vector.memset(warm[:], 0.0)
            nc.scalar.activation(warm[:], warm[:], Act.Sigmoid)

            # ---- precompute txt_hT/img_hT (and temp-net variants) -------
            def proj(lhsT_sb, rhs_sb, n_dc, rhs_w, bias_sb, tag, odt=bf16):
                """out[d, :] = sum_k lhsT[k, d] * rhs[k, :] (+ bias[d])."""
                outs = []
                for dc in range(n_dc):
                    ps = ppre.tile([128, rhs_w], f32, tag="pre", name="pre")
                    for k in range(KC):
                        nc.tensor.matmul(
                            ps[:], lhsT=lhsT_sb[k][:, dc * 128:(dc + 1) * 128],
                            rhs=rhs_sb[k][:], start=(k == 0), stop=(k == KC - 1))
                    o = cp.tile([128, rhs_w], odt, tag=f"{tag}{dc}",
                                name=f"{tag}{dc}")
                    if bias_sb is not None:
                        nc.scalar.activation(o[:], ps[:], Act.Identity,
                                             bias=bias_sb[:, dc:dc + 1])
                    else:
                        nc.scalar.copy(o[:], ps[:])
                    outs.append(o)
                return outs

            txt_tT = proj(wt1bT_sb, txtT_sb, DC_TMP, B, bt1_sb, "txttT")
            img_tT = proj(wt1aT_sb, imgT_sb, DC_TMP, ROWS, None, "imgtT", odt=f32)
            txt_hT = proj(w1bT_sb, txtT_sb, DC_SIM, B, b1_sb, "txthT")
            img_hT = proj(w1aT_sb, imgT_sb, DC_SIM, ROWS, None, "imghT", odt=f32)

            # cosine in permuted strip layout (head PE slack): psum partition
            # 32q+v = output row 4v+q via host-permuted imgTp (carries 7/3)
            cos_ps = pacc.tile([128, B], f32, tag="cos")
            for k in range(KC):
                nc.tensor.matmul(cos_ps[:], lhsT=imgTp_sb[k][:],
                                 rhs=txtT_sb[k][:],
                                 start=(k == 0), stop=(k == KC - 1))

            # ---- main pairwise loops ------------------------------------
            # init with ancillary load: DVE epilogue ~2.8us, ACT precompute
            # copies + sigmoids ~10us
            t_eng = [2800.0, 10000.0]  # dve, act

            def relu_pair(h, src, col):
                if t_eng[0] + _COST_DVE <= t_eng[1] + _COST_ACT:
                    t_eng[0] += _COST_DVE
                    nc.vector.tensor_scalar(h[:], src[:], col, 0.0,
                                            op0=Alu.add, op1=Alu.max)
                else:
                    t_eng[1] += _COST_ACT
                    nc.scalar.activation(h[:], src[:], Act.Relu, bias=col)

            def pair_loop(n_dc, txt_sb, img_sb, s_sb, ps, order):
                for i in order:
                    q, v = i % NQ, i // NQ
                    for c in range(n_dc):
                        h = hp.tile([128, B], bf16, tag="h", name="h")
                        relu_pair(h, txt_sb[c], img_sb[c][:, i:i + 1])
                        nc.tensor.matmul(
                            ps[32 * q:32 * q + 32, :],
                            lhsT=s_sb[c][:, 32 * v:32 * v + 32],
                            rhs=h[:],
                            start=(v == 0 and c == 0),
                            stop=(v == NV - 1 and c == n_dc - 1),
                            tile_position=(0, 32 * q))

            tmp_ps = pacc.tile([128, B], f32, tag="ptmp")
            sim_ps = pacc.tile([128, B], f32, tag="psim")

            # temp path first: its epilogue (sigmoid+affine+reciprocal)
            # overlaps the sim path's main loop.
            pair_loop(DC_TMP, txt_tT, img_tT, s_tmp_sb, tmp_ps, range(ROWS))
            tmp_sb = ep.tile([128, B], f32, tag="tmp")
            nc.scalar.activation(tmp_sb[:], tmp_ps[:], Act.Sigmoid,
                                 bias=bias2_sb[:, 1:2])
            # tmp := (0.2*tmp + 0.01)/0.3 ; rec := 1/tmp
            # (the 0.3 from 0.3*adaptive_sim is folded here; 0.7 on cos is
            # folded into the host-scaled imgTp = 7/3 * img rows)
            nc.vector.tensor_scalar(tmp_sb[:], tmp_sb[:], 2.0 / 3.0, 1.0 / 30.0,
                                    op0=Alu.mult, op1=Alu.add)
            rec_sb = ep.tile([128, B], f32, tag="rec")
            nc.vector.reciprocal_approx_fast(rec_sb[:], tmp_sb[:])

            pair_loop(DC_SIM, txt_hT, img_hT, s_sim_sb, sim_ps, range(ROWS))

            # ---- epilogue (all in permuted strip layout) ----------------
            sim_sb = ep.tile([128, B], f32, tag="sim")
            nc.scalar.activation(sim_sb[:], sim_ps[:], Act.Sigmoid,
                                 bias=bias2_sb[:, 0:1])
            # out = ((7/3)*cos + sim) * (0.3/temp)
            mix_sb = ep.tile([128, B], f32, tag="mix")
            nc.vector.tensor_add(mix_sb[:], cos_ps[:], sim_sb[:])
            out_sb = ep.tile([128, B], f32, tag="outsb")
            nc.vector.tensor_mul(out_sb[:], mix_sb[:], rec_sb[:])
            # un-permute rows on the way out: sbuf partition 32q+v -> row 4v+q
            # (one DMA per strip: multi-dim partition APs don't lower in DMA)
            out_v = out_d.rearrange("(v q) j -> v q j", q=NQ)
            for q in range(NQ):
                eng = nc.sync if q % 2 == 0 else nc.gpsimd
                eng.dma_start(out_v[:, q, :], out_sb[32 * q:32 * q + NV, :])

    nc.compile()
    return nc


def _prep_inputs(img_emb, txt_emb, W1a, W1b, b1, W2, b2,
                 Wt1a, Wt1b, bt1, Wt2, bt2):
    """Host-side layout prep: transposes + delta-expanded reduction weights."""
    import ml_dtypes
    bf = ml_dtypes.bfloat16
    f = np.float32
    txtT = np.ascontiguousarray(np.asarray(txt_emb, f).T).astype(bf)
    w1aT = np.ascontiguousarray(np.asarray(W1a, f).T).astype(bf)
    w1bT = np.ascontiguousarray(np.asarray(W1b, f).T).astype(bf)
    wt1aT = np.ascontiguousarray(np.asarray(Wt1a, f).T).astype(bf)
    wt1bT = np.ascontiguousarray(np.asarray(Wt1b, f).T).astype(bf)
    b1c = np.ascontiguousarray(np.asarray(b1, f).reshape(DC_SIM, 128).T).astype(bf)
    bt1c = np.ascontiguousarray(np.asarray(bt1, f).reshape(DC_TMP, 128).T).astype(bf)

    def delta_expand(w, n_dc):
        w = np.asarray(w, f).reshape(n_dc, 128)
        s = np.zeros((n_dc, 128, NV, 32), f)
        for v in range(NV):
            s[:, :, v, v] = w
        return s.reshape(n_dc, 128, 32 * NV).astype(bf)

    s_sim = delta_expand(W2, DC_SIM)
    s_tmp = delta_expand(Wt2, DC_TMP)
    bias2 = np.empty((128, 2), f)
    bias2[:, 0] = f(b2)
    bias2[:, 1] = f(bt2)

    img = np.asarray(img_emb, f)
    shared = dict(txtT=txtT, w1aT=w1aT, w1bT=w1bT, wt1aT=wt1aT, wt1bT=wt1bT,
                  b1c=b1c, bt1c=bt1c, s_sim=s_sim, s_tmp=s_tmp, bias2=bias2)
    in_maps = []
    for m in range(NCORES):
        im = dict(shared)
        rows = img[m * ROWS:(m + 1) * ROWS, :]          # [64, 512]
        im["imgT"] = np.ascontiguousarray(rows.T).astype(bf)
        # permuted strip layout: column 32q+v = row 4v+q, v<NV; rest zero
        p = np.zeros((D, 128), f)
        for i in range(ROWS):
            q, v = i % NQ, i // NQ
            p[:, 32 * q + v] = (7.0 / 3.0) * rows[i, :]
        im["imgTp"] = p.astype(bf)
        in_maps.append(im)
    return in_maps


def _run(inputs, trace=False, tmpdir=None):
    from concourse.bass_utils import run_bass_kernel_spmd

    if "nc" not in _CACHE:
        _CACHE["nc"] = _build()
    nc = _CACHE["nc"]
    in_maps = _prep_inputs(**inputs)
    res = run_bass_kernel_spmd(nc, in_maps, core_ids=list(range(NCORES)),
                               trace=trace, tmpdir=tmpdir)
    out = np.concatenate([res.results[m]["out"] for m in range(NCORES)], axis=0)
    return out.astype(np.float32), res


def kernel(**inputs):
    out, _ = _run(inputs)
    return out


# revision 20
# speedup vs baseline: 1.0167x; 1.0167x over previous
"""Trainium2 Bass kernel for AdaptiveSimilarityLearning (pairwise MLP gate).

Computes, for B=512 image/text embeddings (D=512):
  img_h = img @ W1a.T ; txt_h = txt @ W1b.T
  adaptive_sim = sigmoid(sum_d relu(img_h[i,d]+txt_h[j,d]+b1[d]) * W2[d] + b2)
  adaptive_temp = 0.01 + 0.2*sigmoid(sum_d relu(img_t+txt_t+bt1)*Wt2 + bt2)
  out = (0.7 * img@txt.T + 0.3*adaptive_sim) / adaptive_temp

Sharding: rows of the B^2 grid split over 8 NeuronCores (64 rows each);
txt-side tensors and weights replicated. No collectives.

Per-core mapping:
  - d lives on SBUF partitions: txt_hT [d, j] tiles of [128, 512] (bf16).
  - relu(txt_hT[d,j] + img_hT[d,i]) is a per-partition-scalar op: DVE
    tensor_scalar(add, max 0) or ACT activation(Relu, bias) per (i, d-chunk).
  - The weighted partition-reduction sum_d h[d,j]*W2[d] runs on the PE as a
    bf16 matmul with a "delta" stationary S[k, 32v+m] = W2c[k]*(m==v).
  - Row i maps to PE column strip q=i%4 (tile_position=(0,32q)) at delta
    position v=i//4, so 4 consecutive rows' matmuls run on 4 independent
    32-column sub-array strips CONCURRENTLY (~54ns/MM effective) and the
    per-MM LDWEIGHTS is hidden. PSUM accumulator [128, 512] holds output
    row i at partition 32*(i%4) + i//4; cosine is computed in the same
    permuted layout via a host-permuted imgT, and the output DMA
    un-permutes rows on the way to DRAM.

All matmul operands are bf16 (fp32 matmuls lower to 2 half-rate passes on
TRN2); PSUM accumulation and the epilogue stay f32.
"""

import numpy as np

B = 512
D = 512
DH = 256
NCORES = 8
ROWS = B // NCORES  # 64 rows of the pairwise grid per core
KC = D // 128       # 4 contraction chunks
DC_SIM = D // 128   # 4 d-chunks (sim path)
DC_TMP = DH // 128  # 2 d-chunks (temp path)
NQ = 4              # PE column strips
NV = ROWS // NQ     # delta positions used per strip (16)

# measured per-[128,512]-tile costs (ns) used for static load balancing
_COST_DVE = 340.0
_COST_ACT = 704.0

_CACHE = {}


def _build():
    import concourse.tile as tile
    from concourse import bacc, mybir

    f32 = mybir.dt.float32
    bf16 = mybir.dt.bfloat16
    Alu = mybir.AluOpType
    Act = mybir.ActivationFunctionType

    nc = bacc.Bacc("TRN2", target_bir_lowering=False, debug=False,
                   num_devices=NCORES)

    dp = lambda name, shape: nc.dram_tensor(name, shape, bf16,
                                            kind="ExternalInput").ap()
    imgT = dp("imgT", [D, ROWS])        # k x i (natural row order)
    imgTp = dp("imgTp", [D, 128])       # k x permuted+padded strip layout
    txtT = dp("txtT", [D, B])
    w1aT = dp("w1aT", [D, D])
    w1bT = dp("w1bT", [D, D])
    wt1aT = dp("wt1aT", [D, DH])
    wt1bT = dp("wt1bT", [D, DH])
    b1c = dp("b1c", [128, DC_SIM])
    bt1c = dp("bt1c", [128, DC_TMP])
    s_sim = dp("s_sim", [DC_SIM, 128, 32 * NV])
    s_tmp = dp("s_tmp", [DC_TMP, 128, 32 * NV])
    bias2 = nc.dram_tensor("bias2", [128, 2], f32, kind="ExternalInput").ap()
    out_d = nc.dram_tensor("out", [ROWS, B], f32, kind="ExternalOutput").ap()

    with tile.TileContext(nc) as tc:
        with (
            tc.tile_pool(name="consts", bufs=1) as cp,
            tc.tile_pool(name="hpool", bufs=16) as hp,
            tc.tile_pool(name="psacc", bufs=1, space="PSUM") as pacc,
            tc.tile_pool(name="pspre", bufs=4, space="PSUM") as ppre,
            tc.tile_pool(name="epi", bufs=1) as ep,
        ):
            # ---- DMA inputs to SBUF -------------------------------------
            dma_rr = [0]

            def dma_in(dst, srcap):
                eng = nc.sync if dma_rr[0] % 2 == 0 else nc.gpsimd
                dma_rr[0] += 1
                eng.dma_start(dst, srcap)

            def load_chunks(ap, n_k, width, tag, dt=bf16):
                ts = []
                for k in range(n_k):
                    t = cp.tile([128, width], dt, tag=f"{tag}{k}",
                                name=f"{tag}{k}")
                    dma_in(t[:], ap[k * 128:(k + 1) * 128, :])
                    ts.append(t)
                return ts

            # temp-path tensors first so the temp projections start ASAP
            wt1bT_sb = load_chunks(wt1bT, KC, DH, "wt1bT")
            txtT_sb = load_chunks(txtT, KC, B, "txtT")
            bt1_sb = cp.tile([128, DC_TMP], bf16, tag="bt1")
            dma_in(bt1_sb[:], bt1c[:, :])
            wt1aT_sb = load_chunks(wt1aT, KC, DH, "wt1aT")
            imgT_sb = load_chunks(imgT, KC, ROWS, "imgT")
            s_tmp_sb = []
            for c in range(DC_TMP):
                t = cp.tile([128, 32 * NV], bf16, tag=f"stmp{c}", name=f"stmp{c}")
                dma_in(t[:], s_tmp[c, :, :])
                s_tmp_sb.append(t)
            w1bT_sb = load_chunks(w1bT, KC, D, "w1bT")
            b1_sb = cp.tile([128, DC_SIM], bf16, tag="b1")
            dma_in(b1_sb[:], b1c[:, :])
            w1aT_sb = load_chunks(w1aT, KC, D, "w1aT")
            s_sim_sb = []
            for c in range(DC_SIM):
                t = cp.tile([128, 32 * NV], bf16, tag=f"ssim{c}", name=f"ssim{c}")
                dma_in(t[:], s_sim[c, :, :])
                s_sim_sb.append(t)
            imgTp_sb = load_chunks(imgTp, KC, 128, "imgTp")
            bias2_sb = cp.tile([128, 2], f32, tag="bias2")
            dma_in(bias2_sb[:], bias2[:, :])

            # warm the ACT sigmoid table set during the DMA head
            warm = cp.tile([1, 1], f32, tag="warm")
            nc.vector.memset(warm[:], 0.0)
            nc.scalar.activation(warm[:], warm[:], Act.Sigmoid)

            # ---- precompute txt_hT/img_hT (and temp-net variants) -------
            def proj(lhsT_sb, rhs_sb, n_dc, rhs_w, bias_sb, tag, odt=bf16):
                """out[d, :] = sum_k lhsT[k, d] * rhs[k, :] (+ bias[d])."""
                outs = []
                for dc in range(n_dc):
                    ps = ppre.tile([128, rhs_w], f32, tag="pre", name="pre")
                    for k in range(KC):
                        nc.tensor.matmul(
                            ps[:], lhsT=lhsT_sb[k][:, dc * 128:(dc + 1) * 128],
                            rhs=rhs_sb[k][:], start=(k == 0), stop=(k == KC - 1))
                    o = cp.tile([128, rhs_w], odt, tag=f"{tag}{dc}",
                                name=f"{tag}{dc}")
                    if bias_sb is not None:
                        nc.scalar.activation(o[:], ps[:], Act.Identity,
                                             bias=bias_sb[:, dc:dc + 1])
                    else:
                        nc.scalar.copy(o[:], ps[:])
                    outs.append(o)
                return outs

            txt_tT = proj(wt1bT_sb, txtT_sb, DC_TMP, B, bt1_sb, "txttT")
            img_tT = proj(wt1aT_sb, imgT_sb, DC_TMP, ROWS, None, "imgtT", odt=f32)
            txt_hT = proj(w1bT_sb, txtT_sb, DC_SIM, B, b1_sb, "txthT")
            img_hT = proj(w1aT_sb, imgT_sb, DC_SIM, ROWS, None, "imghT", odt=f32)

            # ---- main pairwise loops ------------------------------------
            # init with ancillary load: DVE epilogue ~2.8us, ACT precompute
            # copies + sigmoids ~10us
            t_eng = [2800.0, 10000.0]  # dve, act

            def relu_pair(h, src, col):
                if t_eng[0] + _COST_DVE <= t_eng[1] + _COST_ACT:
                    t_eng[0] += _COST_DVE
                    nc.vector.tensor_scalar(h[:], src[:], col, 0.0,
                                            op0=Alu.add, op1=Alu.max)
                else:
                    t_eng[1] += _COST_ACT
                    nc.scalar.activation(h[:], src[:], Act.Relu, bias=col)

            def pair_loop(n_dc, txt_sb, img_sb, s_sb, ps, order):
                for i in order:
                    q, v = i % NQ, i // NQ
                    for c in range(n_dc):
                        h = hp.tile([128, B], bf16, tag="h", name="h")
                        relu_pair(h, txt_sb[c], img_sb[c][:, i:i + 1])
                        nc.tensor.matmul(
                            ps[32 * q:32 * q + 32, :],
                            lhsT=s_sb[c][:, 32 * v:32 * v + 32],
                            rhs=h[:],
                            start=(v == 0 and c == 0),
                            stop=(v == NV - 1 and c == n_dc - 1),
                            tile_position=(0, 32 * q))

            tmp_ps = pacc.tile([128, B], f32, tag="ptmp")
            sim_ps = pacc.tile([128, B], f32, tag="psim")

            # temp path first: its epilogue (sigmoid+affine+reciprocal)
            # overlaps the sim path's main loop.
            pair_loop(DC_TMP, txt_tT, img_tT, s_tmp_sb, tmp_ps, range(ROWS))
            tmp_sb = ep.tile([128, B], f32, tag="tmp")
            nc.scalar.activation(tmp_sb[:], tmp_ps[:], Act.Sigmoid,
                                 bias=bias2_sb[:, 1:2])
            # tmp := (0.2*tmp + 0.01)/0.3 ; rec := 1/tmp
            # (the 0.3 from 0.3*adaptive_sim is folded here; 0.7 on cos is
            # folded into the host-scaled imgTp = 7/3 * img rows)
            nc.vector.tensor_scalar(tmp_sb[:], tmp_sb[:], 2.0 / 3.0, 1.0 / 30.0,
                                    op0=Alu.mult, op1=Alu.add)
            rec_sb = ep.tile([128, B], f32, tag="rec")
            nc.vector.reciprocal_approx_fast(rec_sb[:], tmp_sb[:])

            # cosine in permuted strip layout, between the two pair loops
            # (imgTp has landed; keeps it off both the PE head and the tail):
            # psum partition 32q+v = output row 4v+q via host-permuted imgTp,
            # which also carries the 7/3 mix factor
            cos_ps = pacc.tile([128, B], f32, tag="cos")
            for k in range(KC):
                nc.tensor.matmul(cos_ps[:], lhsT=imgTp_sb[k][:],
                                 rhs=txtT_sb[k][:],
                                 start=(k == 0), stop=(k == KC - 1))

            pair_loop(DC_SIM, txt_hT, img_hT, s_sim_sb, sim_ps, range(ROWS))

            # ---- epilogue (all in permuted strip layout) ----------------
            sim_sb = ep.tile([128, B], f32, tag="sim")
            nc.scalar.activation(sim_sb[:], sim_ps[:], Act.Sigmoid,
                                 bias=bias2_sb[:, 0:1])
            # out = ((7/3)*cos + sim) * (0.3/temp)
            mix_sb = ep.tile([128, B], f32, tag="mix")
            nc.vector.tensor_add(mix_sb[:], cos_ps[:], sim_sb[:])
            out_sb = ep.tile([128, B], f32, tag="outsb")
            nc.vector.tensor_mul(out_sb[:], mix_sb[:], rec_sb[:])
            # un-permute rows on the way out: sbuf partition 32q+v -> row 4v+q
            # (one DMA per strip: multi-dim partition APs don't lower in DMA)
            out_v = out_d.rearrange("(v q) j -> v q j", q=NQ)
            for q in range(NQ):
                eng = nc.sync if q % 2 == 0 else nc.gpsimd
                eng.dma_start(out_v[:, q, :], out_sb[32 * q:32 * q + NV, :])

    nc.compile()
    return nc


def _prep_inputs(img_emb, txt_emb, W1a, W1b, b1, W2, b2,
                 Wt1a, Wt1b, bt1, Wt2, bt2):
    """Host-side layout prep: transposes + delta-expanded reduction weights."""
    import ml_dtypes
    bf = ml_dtypes.bfloat16
    f = np.float32
    txtT = np.ascontiguousarray(np.asarray(txt_emb, f).T).astype(bf)
    w1aT = np.ascontiguousarray(np.asarray(W1a, f).T).astype(bf)
    w1bT = np.ascontiguousarray(np.asarray(W1b, f).T).astype(bf)
    wt1aT = np.ascontiguousarray(np.asarray(Wt1a, f).T).astype(bf)
    wt1bT = np.ascontiguousarray(np.asarray(Wt1b, f).T).astype(bf)
    b1c = np.ascontiguousarray(np.asarray(b1, f).reshape(DC_SIM, 128).T).astype(bf)
    bt1c = np.ascontiguousarray(np.asarray(bt1, f).reshape(DC_TMP, 128).T).astype(bf)

    def delta_expand(w, n_dc):
        w = np.asarray(w, f).reshape(n_dc, 128)
        s = np.zeros((n_dc, 128, NV, 32), f)
        for v in range(NV):
            s[:, :, v, v] = w
        return s.reshape(n_dc, 128, 32 * NV).astype(bf)

    s_sim = delta_expand(W2, DC_SIM)
    s_tmp = delta_expand(Wt2, DC_TMP)
    bias2 = np.empty((128, 2), f)
    bias2[:, 0] = f(b2)
    bias2[:, 1] = f(bt2)

    img = np.asarray(img_emb, f)
    shared = dict(txtT=txtT, w1aT=w1aT, w1bT=w1bT, wt1aT=wt1aT, wt1bT=wt1bT,
                  b1c=b1c, bt1c=bt1c, s_sim=s_sim, s_tmp=s_tmp, bias2=bias2)
    in_maps = []
    for m in range(NCORES):
        im = dict(shared)
        rows = img[m * ROWS:(m + 1) * ROWS, :]          # [64, 512]
        im["imgT"] = np.ascontiguousarray(rows.T).astype(bf)
        # permuted strip layout: column 32q+v = row 4v+q, v<NV; rest zero
        p = np.zeros((D, 128), f)
        for i in range(ROWS):
            q, v = i % NQ, i // NQ
            p[:, 32 * q + v] = (7.0 / 3.0) * rows[i, :]
        im["imgTp"] = p.astype(bf)
        in_maps.append(im)
    return in_maps


def _run(inputs, trace=False, tmpdir=None):
    from concourse.bass_utils import run_bass_kernel_spmd

    if "nc" not in _CACHE:
        _CACHE["nc"] = _build()
    nc = _CACHE["nc"]
    in_maps = _prep_inputs(**inputs)
    res = run_bass_kernel_spmd(nc, in_maps, core_ids=list(range(NCORES)),
                               trace=trace, tmpdir=tmpdir)
    out = np.concatenate([res.results[m]["out"] for m in range(NCORES)], axis=0)
    return out.astype(np.float32), res


def kernel(**inputs):
    out, _ = _run(inputs)
    return out


# revision 21
# speedup vs baseline: 1.0237x; 1.0069x over previous
"""Trainium2 Bass kernel for AdaptiveSimilarityLearning (pairwise MLP gate).

Computes, for B=512 image/text embeddings (D=512):
  img_h = img @ W1a.T ; txt_h = txt @ W1b.T
  adaptive_sim = sigmoid(sum_d relu(img_h[i,d]+txt_h[j,d]+b1[d]) * W2[d] + b2)
  adaptive_temp = 0.01 + 0.2*sigmoid(sum_d relu(img_t+txt_t+bt1)*Wt2 + bt2)
  out = (0.7 * img@txt.T + 0.3*adaptive_sim) / adaptive_temp

Sharding: rows of the B^2 grid split over 8 NeuronCores (64 rows each);
txt-side tensors and weights replicated. No collectives.

Per-core mapping:
  - d lives on SBUF partitions: txt_hT [d, j] tiles of [128, 512] (bf16).
  - relu(txt_hT[d,j] + img_hT[d,i]) is a per-partition-scalar op: DVE
    tensor_scalar(add, max 0) or ACT activation(Relu, bias) per (i, d-chunk).
  - The weighted partition-reduction sum_d h[d,j]*W2[d] runs on the PE as a
    bf16 matmul with a "delta" stationary S[k, 32v+m] = W2c[k]*(m==v).
  - Row i maps to PE column strip q=i%4 (tile_position=(0,32q)) at delta
    position v=i//4, so 4 consecutive rows' matmuls run on 4 independent
    32-column sub-array strips CONCURRENTLY (~54ns/MM effective) and the
    per-MM LDWEIGHTS is hidden. PSUM accumulator [128, 512] holds output
    row i at partition 32*(i%4) + i//4; cosine is computed in the same
    permuted layout via a host-permuted imgT, and the output DMA
    un-permutes rows on the way to DRAM.

All matmul operands are bf16 (fp32 matmuls lower to 2 half-rate passes on
TRN2); PSUM accumulation and the epilogue stay f32.
"""

import numpy as np

B = 512
D = 512
DH = 256
NCORES = 8
ROWS = B // NCORES  # 64 rows of the pairwise grid per core
KC = D // 128       # 4 contraction chunks
DC_SIM = D // 128   # 4 d-chunks (sim path)
DC_TMP = DH // 128  # 2 d-chunks (temp path)
NQ = 4              # PE column strips
NV = ROWS // NQ     # delta positions used per strip (16)

# measured per-[128,512]-tile costs (ns) used for static load balancing
_COST_DVE = 340.0
_COST_ACT = 704.0

_CACHE = {}


def _build():
    import concourse.tile as tile
    from concourse import bacc, mybir

    f32 = mybir.dt.float32
    bf16 = mybir.dt.bfloat16
    Alu = mybir.AluOpType
    Act = mybir.ActivationFunctionType

    nc = bacc.Bacc("TRN2", target_bir_lowering=False, debug=False,
                   num_devices=NCORES)

    dp = lambda name, shape: nc.dram_tensor(name, shape, bf16,
                                            kind="ExternalInput").ap()
    imgT = dp("imgT", [D, ROWS])        # k x i (natural row order)
    imgTp = dp("imgTp", [D, 128])       # k x permuted+padded strip layout
    txtT = dp("txtT", [D, B])
    w1aT = dp("w1aT", [D, D])
    w1bT = dp("w1bT", [D, D])
    wt1aT = dp("wt1aT", [D, DH])
    wt1bT = dp("wt1bT", [D, DH])
    b1c = dp("b1c", [128, DC_SIM])
    bt1c = dp("bt1c", [128, DC_TMP])
    s_sim = dp("s_sim", [DC_SIM, 128, 32 * NV])
    s_tmp = dp("s_tmp", [DC_TMP, 128, 32 * NV])
    bias2 = nc.dram_tensor("bias2", [128, 2], f32, kind="ExternalInput").ap()
    out_d = nc.dram_tensor("out", [ROWS, B], f32, kind="ExternalOutput").ap()

    with tile.TileContext(nc) as tc:
        with (
            tc.tile_pool(name="consts", bufs=1) as cp,
            tc.tile_pool(name="hpool", bufs=16) as hp,
            tc.tile_pool(name="psacc", bufs=1, space="PSUM") as pacc,
            tc.tile_pool(name="pspre", bufs=4, space="PSUM") as ppre,
            tc.tile_pool(name="epi", bufs=1) as ep,
        ):
            # ---- DMA inputs to SBUF -------------------------------------
            dma_rr = [0]

            def dma_in(dst, srcap):
                eng = nc.sync if dma_rr[0] % 2 == 0 else nc.gpsimd
                dma_rr[0] += 1
                eng.dma_start(dst, srcap)

            def load_chunks(ap, n_k, width, tag, dt=bf16):
                ts = []
                for k in range(n_k):
                    t = cp.tile([128, width], dt, tag=f"{tag}{k}",
                                name=f"{tag}{k}")
                    dma_in(t[:], ap[k * 128:(k + 1) * 128, :])
                    ts.append(t)
                return ts

            # temp-path tensors first so the temp projections start ASAP
            wt1bT_sb = load_chunks(wt1bT, KC, DH, "wt1bT")
            txtT_sb = load_chunks(txtT, KC, B, "txtT")
            bt1_sb = cp.tile([128, DC_TMP], bf16, tag="bt1")
            dma_in(bt1_sb[:], bt1c[:, :])
            wt1aT_sb = load_chunks(wt1aT, KC, DH, "wt1aT")
            imgT_sb = load_chunks(imgT, KC, ROWS, "imgT")
            s_tmp_sb = []
            for c in range(DC_TMP):
                t = cp.tile([128, 32 * NV], bf16, tag=f"stmp{c}", name=f"stmp{c}")
                dma_in(t[:], s_tmp[c, :, :])
                s_tmp_sb.append(t)
            w1bT_sb = load_chunks(w1bT, KC, D, "w1bT")
            b1_sb = cp.tile([128, DC_SIM], bf16, tag="b1")
            dma_in(b1_sb[:], b1c[:, :])
            w1aT_sb = load_chunks(w1aT, KC, D, "w1aT")
            s_sim_sb = []
            for c in range(DC_SIM):
                t = cp.tile([128, 32 * NV], bf16, tag=f"ssim{c}", name=f"ssim{c}")
                dma_in(t[:], s_sim[c, :, :])
                s_sim_sb.append(t)
            imgTp_sb = load_chunks(imgTp, KC, 128, "imgTp")
            bias2_sb = cp.tile([128, 2], f32, tag="bias2")
            dma_in(bias2_sb[:], bias2[:, :])

            # warm the ACT sigmoid table set during the DMA head
            warm = cp.tile([1, 1], f32, tag="warm")
            nc.vector.memset(warm[:], 0.0)
            nc.scalar.activation(warm[:], warm[:], Act.Sigmoid)

            # ---- precompute txt_hT/img_hT (and temp-net variants) -------
            def proj(lhsT_sb, rhs_sb, n_dc, rhs_w, bias_sb, tag, odt=bf16):
                """out[d, :] = sum_k lhsT[k, d] * rhs[k, :] (+ bias[d])."""
                outs = []
                for dc in range(n_dc):
                    ps = ppre.tile([128, rhs_w], f32, tag="pre", name="pre")
                    for k in range(KC):
                        nc.tensor.matmul(
                            ps[:], lhsT=lhsT_sb[k][:, dc * 128:(dc + 1) * 128],
                            rhs=rhs_sb[k][:], start=(k == 0), stop=(k == KC - 1))
                    o = cp.tile([128, rhs_w], odt, tag=f"{tag}{dc}",
                                name=f"{tag}{dc}")
                    if bias_sb is not None:
                        nc.scalar.activation(o[:], ps[:], Act.Identity,
                                             bias=bias_sb[:, dc:dc + 1])
                    else:
                        nc.scalar.copy(o[:], ps[:])
                    outs.append(o)
                return outs

            txt_tT = proj(wt1bT_sb, txtT_sb, DC_TMP, B, bt1_sb, "txttT")
            img_tT = proj(wt1aT_sb, imgT_sb, DC_TMP, ROWS, None, "imgtT", odt=f32)
            txt_hT = proj(w1bT_sb, txtT_sb, DC_SIM, B, b1_sb, "txthT")
            img_hT = proj(w1aT_sb, imgT_sb, DC_SIM, ROWS, None, "imghT", odt=f32)

            # ---- main pairwise loops ------------------------------------
            # init with ancillary load: DVE epilogue ~2.8us, ACT precompute
            # copies + sigmoids ~10us
            t_eng = [2800.0, 10000.0]  # dve, act

            def relu_pair(h, src, col):
                if t_eng[0] + _COST_DVE <= t_eng[1] + _COST_ACT:
                    t_eng[0] += _COST_DVE
                    nc.vector.tensor_scalar(h[:], src[:], col, 0.0,
                                            op0=Alu.add, op1=Alu.max)
                else:
                    t_eng[1] += _COST_ACT
                    nc.scalar.activation(h[:], src[:], Act.Relu, bias=col)

            def pair_loop(n_dc, txt_sb, img_sb, s_sb, ps, order):
                for i in order:
                    q, v = i % NQ, i // NQ
                    for c in range(n_dc):
                        h = hp.tile([128, B], bf16, tag="h", name="h")
                        relu_pair(h, txt_sb[c], img_sb[c][:, i:i + 1])
                        nc.tensor.matmul(
                            ps[32 * q:32 * q + 32, :],
                            lhsT=s_sb[c][:, 32 * v:32 * v + 32],
                            rhs=h[:],
                            start=(v == 0 and c == 0),
                            stop=(v == NV - 1 and c == n_dc - 1),
                            tile_position=(0, 32 * q))

            tmp_ps = pacc.tile([128, B], f32, tag="ptmp")
            sim_ps = pacc.tile([128, B], f32, tag="psim")

            # temp path first: its epilogue (sigmoid+affine+reciprocal)
            # overlaps the sim path's main loop.
            pair_loop(DC_TMP, txt_tT, img_tT, s_tmp_sb, tmp_ps, range(ROWS))
            tmp_sb = ep.tile([128, B], f32, tag="tmp")
            nc.scalar.activation(tmp_sb[:], tmp_ps[:], Act.Sigmoid,
                                 bias=bias2_sb[:, 1:2])
            # tmp := (0.2*tmp + 0.01)/0.3 ; rec := 1/tmp
            # (the 0.3 from 0.3*adaptive_sim is folded here; 0.7 on cos is
            # folded into the host-scaled imgTp = 7/3 * img rows)
            nc.vector.tensor_scalar(tmp_sb[:], tmp_sb[:], 2.0 / 3.0, 1.0 / 30.0,
                                    op0=Alu.mult, op1=Alu.add)
            rec_sb = ep.tile([128, B], f32, tag="rec")
            nc.vector.reciprocal_approx_fast(rec_sb[:], tmp_sb[:])

            pair_loop(DC_SIM, txt_hT, img_hT, s_sim_sb, sim_ps, range(ROWS))

            # ---- cosine in permuted strip layout (PE tail work) ---------
            # psum partition 32q+v = output row 4v+q (via host-permuted imgTp,
            # which also carries the 7/3 mix factor)
            cos_ps = pacc.tile([128, B], f32, tag="cos")
            for k in range(KC):
                nc.tensor.matmul(cos_ps[:], lhsT=imgTp_sb[k][:],
                                 rhs=txtT_sb[k][:],
                                 start=(k == 0), stop=(k == KC - 1))

            # ---- epilogue (all in permuted strip layout) ----------------
            sim_sb = ep.tile([128, B], f32, tag="sim")
            nc.scalar.activation(sim_sb[:], sim_ps[:], Act.Sigmoid,
                                 bias=bias2_sb[:, 0:1])
            # out = ((7/3)*cos + sim) * (0.3/temp)
            mix_sb = ep.tile([128, B], f32, tag="mix")
            nc.vector.tensor_add(mix_sb[:], cos_ps[:], sim_sb[:])
            out_sb = ep.tile([128, B], f32, tag="outsb")
            nc.vector.tensor_mul(out_sb[:], mix_sb[:], rec_sb[:])
            # un-permute rows on the way out: sbuf partition 32q+v -> row 4v+q
            # (one DMA per strip: multi-dim partition APs don't lower in DMA)
            out_v = out_d.rearrange("(v q) j -> v q j", q=NQ)
            for q in range(NQ):
                eng = nc.sync if q % 2 == 0 else nc.gpsimd
                eng.dma_start(out_v[:, q, :], out_sb[32 * q:32 * q + NV, :])

    nc.compile()
    return nc


def _prep_inputs(img_emb, txt_emb, W1a, W1b, b1, W2, b2,
                 Wt1a, Wt1b, bt1, Wt2, bt2):
    """Host-side layout prep: transposes + delta-expanded reduction weights."""
    import ml_dtypes
    bf = ml_dtypes.bfloat16
    f = np.float32
    txtT = np.ascontiguousarray(np.asarray(txt_emb, f).T).astype(bf)
    w1aT = np.ascontiguousarray(np.asarray(W1a, f).T).astype(bf)
    w1bT = np.ascontiguousarray(np.asarray(W1b, f).T).astype(bf)
    wt1aT = np.ascontiguousarray(np.asarray(Wt1a, f).T).astype(bf)
    wt1bT = np.ascontiguousarray(np.asarray(Wt1b, f).T).astype(bf)
    b1c = np.ascontiguousarray(np.asarray(b1, f).reshape(DC_SIM, 128).T).astype(bf)
    bt1c = np.ascontiguousarray(np.asarray(bt1, f).reshape(DC_TMP, 128).T).astype(bf)

    def delta_expand(w, n_dc):
        w = np.asarray(w, f).reshape(n_dc, 128)
        s = np.zeros((n_dc, 128, NV, 32), f)
        for v in range(NV):
            s[:, :, v, v] = w
        return s.reshape(n_dc, 128, 32 * NV).astype(bf)

    s_sim = delta_expand(W2, DC_SIM)
    s_tmp = delta_expand(Wt2, DC_TMP)
    bias2 = np.empty((128, 2), f)
    bias2[:, 0] = f(b2)
    bias2[:, 1] = f(bt2)

    img = np.asarray(img_emb, f)
    shared = dict(txtT=txtT, w1aT=w1aT, w1bT=w1bT, wt1aT=wt1aT, wt1bT=wt1bT,
                  b1c=b1c, bt1c=bt1c, s_sim=s_sim, s_tmp=s_tmp, bias2=bias2)
    in_maps = []
    for m in range(NCORES):
        im = dict(shared)
        rows = img[m * ROWS:(m + 1) * ROWS, :]          # [64, 512]
        im["imgT"] = np.ascontiguousarray(rows.T).astype(bf)
        # permuted strip layout: column 32q+v = row 4v+q, v<NV; rest zero
        p = np.zeros((D, 128), f)
        for i in range(ROWS):
            q, v = i % NQ, i // NQ
            p[:, 32 * q + v] = (7.0 / 3.0) * rows[i, :]
        im["imgTp"] = p.astype(bf)
        in_maps.append(im)
    return in_maps


def _run(inputs, trace=False, tmpdir=None):
    from concourse.bass_utils import run_bass_kernel_spmd

    if "nc" not in _CACHE:
        _CACHE["nc"] = _build()
    nc = _CACHE["nc"]
    in_maps = _prep_inputs(**inputs)
    res = run_bass_kernel_spmd(nc, in_maps, core_ids=list(range(NCORES)),
                               trace=trace, tmpdir=tmpdir)
    out = np.concatenate([res.results[m]["out"] for m in range(NCORES)], axis=0)
    return out.astype(np.float32), res


def kernel(**inputs):
    out, _ = _run(inputs)
    return out


# revision 22
# speedup vs baseline: 1.0316x; 1.0078x over previous
"""Trainium2 Bass kernel for AdaptiveSimilarityLearning (pairwise MLP gate).

Computes, for B=512 image/text embeddings (D=512):
  img_h = img @ W1a.T ; txt_h = txt @ W1b.T
  adaptive_sim = sigmoid(sum_d relu(img_h[i,d]+txt_h[j,d]+b1[d]) * W2[d] + b2)
  adaptive_temp = 0.01 + 0.2*sigmoid(sum_d relu(img_t+txt_t+bt1)*Wt2 + bt2)
  out = (0.7 * img@txt.T + 0.3*adaptive_sim) / adaptive_temp

Sharding: rows of the B^2 grid split over 8 NeuronCores (64 rows each);
txt-side tensors and weights replicated. No collectives.

Per-core mapping:
  - d lives on SBUF partitions: txt_hT [d, j] tiles of [128, 512] (bf16).
  - relu(txt_hT[d,j] + img_hT[d,i]) is a per-partition-scalar op: DVE
    tensor_scalar(add, max 0) or ACT activation(Relu, bias) per (i, d-chunk).
  - The weighted partition-reduction sum_d h[d,j]*W2[d] runs on the PE as a
    bf16 matmul with a "delta" stationary S[k, 32v+m] = W2c[k]*(m==v).
  - Row i maps to PE column strip q=i%4 (tile_position=(0,32q)) at delta
    position v=i//4, so 4 consecutive rows' matmuls run on 4 independent
    32-column sub-array strips CONCURRENTLY (~54ns/MM effective) and the
    per-MM LDWEIGHTS is hidden. PSUM accumulator [128, 512] holds output
    row i at partition 32*(i%4) + i//4; cosine is computed in the same
    permuted layout via a host-permuted imgT, and the output DMA
    un-permutes rows on the way to DRAM.

All matmul operands are bf16 (fp32 matmuls lower to 2 half-rate passes on
TRN2); PSUM accumulation and the epilogue stay f32.
"""

import numpy as np

B = 512
D = 512
DH = 256
NCORES = 8
ROWS = B // NCORES  # 64 rows of the pairwise grid per core
KC = D // 128       # 4 contraction chunks
DC_SIM = D // 128   # 4 d-chunks (sim path)
DC_TMP = DH // 128  # 2 d-chunks (temp path)
NQ = 4              # PE column strips
NV = ROWS // NQ     # delta positions used per strip (16)

# measured per-[128,512]-tile costs (ns) used for static load balancing
_COST_DVE = 340.0
_COST_ACT = 704.0

_CACHE = {}


def _build():
    import concourse.tile as tile
    from concourse import bacc, mybir

    f32 = mybir.dt.float32
    bf16 = mybir.dt.bfloat16
    Alu = mybir.AluOpType
    Act = mybir.ActivationFunctionType

    nc = bacc.Bacc("TRN2", target_bir_lowering=False, debug=False,
                   num_devices=NCORES)

    dp = lambda name, shape: nc.dram_tensor(name, shape, bf16,
                                            kind="ExternalInput").ap()
    imgT = dp("imgT", [D, ROWS])        # k x i (natural row order)
    imgTp = dp("imgTp", [D, 128])       # k x permuted+padded strip layout
    txtT = dp("txtT", [D, B])
    w1aT = dp("w1aT", [D, D])
    w1bT = dp("w1bT", [D, D])
    wt1aT = dp("wt1aT", [D, DH])
    wt1bT = dp("wt1bT", [D, DH])
    b1c = dp("b1c", [128, DC_SIM])
    bt1c = dp("bt1c", [128, DC_TMP])
    s_sim = dp("s_sim", [DC_SIM, 128, 32 * NV])
    s_tmp = dp("s_tmp", [DC_TMP, 128, 32 * NV])
    bias2 = nc.dram_tensor("bias2", [128, 2], f32, kind="ExternalInput").ap()
    out_d = nc.dram_tensor("out", [ROWS, B], f32, kind="ExternalOutput").ap()

    with tile.TileContext(nc) as tc:
        with (
            tc.tile_pool(name="consts", bufs=1) as cp,
            tc.tile_pool(name="hpool", bufs=24) as hp,
            tc.tile_pool(name="psacc", bufs=1, space="PSUM") as pacc,
            tc.tile_pool(name="pspre", bufs=4, space="PSUM") as ppre,
            tc.tile_pool(name="epi", bufs=1) as ep,
        ):
            # ---- DMA inputs to SBUF -------------------------------------
            dma_rr = [0]

            def dma_in(dst, srcap):
                eng = nc.sync if dma_rr[0] % 2 == 0 else nc.gpsimd
                dma_rr[0] += 1
                eng.dma_start(dst, srcap)

            def load_chunks(ap, n_k, width, tag, dt=bf16):
                ts = []
                for k in range(n_k):
                    t = cp.tile([128, width], dt, tag=f"{tag}{k}",
                                name=f"{tag}{k}")
                    dma_in(t[:], ap[k * 128:(k + 1) * 128, :])
                    ts.append(t)
                return ts

            # temp-path tensors first so the temp projections start ASAP
            wt1bT_sb = load_chunks(wt1bT, KC, DH, "wt1bT")
            txtT_sb = load_chunks(txtT, KC, B, "txtT")
            bt1_sb = cp.tile([128, DC_TMP], bf16, tag="bt1")
            dma_in(bt1_sb[:], bt1c[:, :])
            wt1aT_sb = load_chunks(wt1aT, KC, DH, "wt1aT")
            imgT_sb = load_chunks(imgT, KC, ROWS, "imgT")
            s_tmp_sb = []
            for c in range(DC_TMP):
                t = cp.tile([128, 32 * NV], bf16, tag=f"stmp{c}", name=f"stmp{c}")
                dma_in(t[:], s_tmp[c, :, :])
                s_tmp_sb.append(t)
            w1bT_sb = load_chunks(w1bT, KC, D, "w1bT")
            b1_sb = cp.tile([128, DC_SIM], bf16, tag="b1")
            dma_in(b1_sb[:], b1c[:, :])
            w1aT_sb = load_chunks(w1aT, KC, D, "w1aT")
            s_sim_sb = []
            for c in range(DC_SIM):
                t = cp.tile([128, 32 * NV], bf16, tag=f"ssim{c}", name=f"ssim{c}")
                dma_in(t[:], s_sim[c, :, :])
                s_sim_sb.append(t)
            imgTp_sb = load_chunks(imgTp, KC, 128, "imgTp")
            bias2_sb = cp.tile([128, 2], f32, tag="bias2")
            dma_in(bias2_sb[:], bias2[:, :])

            # warm the ACT sigmoid table set during the DMA head
            warm = cp.tile([1, 1], f32, tag="warm")
            nc.vector.memset(warm[:], 0.0)
            nc.scalar.activation(warm[:], warm[:], Act.Sigmoid)

            # ---- precompute txt_hT/img_hT (and temp-net variants) -------
            def proj(lhsT_sb, rhs_sb, n_dc, rhs_w, bias_sb, tag, odt=bf16):
                """out[d, :] = sum_k lhsT[k, d] * rhs[k, :] (+ bias[d])."""
                outs = []
                for dc in range(n_dc):
                    ps = ppre.tile([128, rhs_w], f32, tag="pre", name="pre")
                    for k in range(KC):
                        nc.tensor.matmul(
                            ps[:], lhsT=lhsT_sb[k][:, dc * 128:(dc + 1) * 128],
                            rhs=rhs_sb[k][:], start=(k == 0), stop=(k == KC - 1))
                    o = cp.tile([128, rhs_w], odt, tag=f"{tag}{dc}",
                                name=f"{tag}{dc}")
                    if bias_sb is not None:
                        nc.scalar.activation(o[:], ps[:], Act.Identity,
                                             bias=bias_sb[:, dc:dc + 1])
                    else:
                        nc.scalar.copy(o[:], ps[:])
                    outs.append(o)
                return outs

            txt_tT = proj(wt1bT_sb, txtT_sb, DC_TMP, B, bt1_sb, "txttT")
            img_tT = proj(wt1aT_sb, imgT_sb, DC_TMP, ROWS, None, "imgtT", odt=f32)
            txt_hT = proj(w1bT_sb, txtT_sb, DC_SIM, B, b1_sb, "txthT")
            img_hT = proj(w1aT_sb, imgT_sb, DC_SIM, ROWS, None, "imghT", odt=f32)

            # ---- main pairwise loops ------------------------------------
            # init with ancillary load: DVE epilogue ~2.8us, ACT precompute
            # copies + sigmoids ~10us
            t_eng = [2800.0, 10000.0]  # dve, act

            def relu_pair(h, src, col):
                if t_eng[0] + _COST_DVE <= t_eng[1] + _COST_ACT:
                    t_eng[0] += _COST_DVE
                    nc.vector.tensor_scalar(h[:], src[:], col, 0.0,
                                            op0=Alu.add, op1=Alu.max)
                else:
                    t_eng[1] += _COST_ACT
                    nc.scalar.activation(h[:], src[:], Act.Relu, bias=col)

            def pair_loop(n_dc, txt_sb, img_sb, s_sb, ps, order):
                for i in order:
                    q, v = i % NQ, i // NQ
                    for c in range(n_dc):
                        h = hp.tile([128, B], bf16, tag="h", name="h")
                        relu_pair(h, txt_sb[c], img_sb[c][:, i:i + 1])
                        nc.tensor.matmul(
                            ps[32 * q:32 * q + 32, :],
                            lhsT=s_sb[c][:, 32 * v:32 * v + 32],
                            rhs=h[:],
                            start=(v == 0 and c == 0),
                            stop=(v == NV - 1 and c == n_dc - 1),
                            tile_position=(0, 32 * q))

            tmp_ps = pacc.tile([128, B], f32, tag="ptmp")
            sim_ps = pacc.tile([128, B], f32, tag="psim")

            # temp path first: its epilogue (sigmoid+affine+reciprocal)
            # overlaps the sim path's main loop.
            pair_loop(DC_TMP, txt_tT, img_tT, s_tmp_sb, tmp_ps, range(ROWS))
            tmp_sb = ep.tile([128, B], f32, tag="tmp")
            nc.scalar.activation(tmp_sb[:], tmp_ps[:], Act.Sigmoid,
                                 bias=bias2_sb[:, 1:2])
            # tmp := (0.2*tmp + 0.01)/0.3 ; rec := 1/tmp
            # (the 0.3 from 0.3*adaptive_sim is folded here; 0.7 on cos is
            # folded into the host-scaled imgTp = 7/3 * img rows)
            nc.vector.tensor_scalar(tmp_sb[:], tmp_sb[:], 2.0 / 3.0, 1.0 / 30.0,
                                    op0=Alu.mult, op1=Alu.add)
            rec_sb = ep.tile([128, B], f32, tag="rec")
            nc.vector.reciprocal_approx_fast(rec_sb[:], tmp_sb[:])

            pair_loop(DC_SIM, txt_hT, img_hT, s_sim_sb, sim_ps, range(ROWS))

            # ---- cosine in permuted strip layout (PE tail work) ---------
            # psum partition 32q+v = output row 4v+q (via host-permuted imgTp,
            # which also carries the 7/3 mix factor)
            cos_ps = pacc.tile([128, B], f32, tag="cos")
            for k in range(KC):
                nc.tensor.matmul(cos_ps[:], lhsT=imgTp_sb[k][:],
                                 rhs=txtT_sb[k][:],
                                 start=(k == 0), stop=(k == KC - 1))

            # ---- epilogue (all in permuted strip layout) ----------------
            sim_sb = ep.tile([128, B], f32, tag="sim")
            nc.scalar.activation(sim_sb[:], sim_ps[:], Act.Sigmoid,
                                 bias=bias2_sb[:, 0:1])
            # out = ((7/3)*cos + sim) * (0.3/temp)
            mix_sb = ep.tile([128, B], f32, tag="mix")
            nc.vector.tensor_add(mix_sb[:], cos_ps[:], sim_sb[:])
            out_sb = ep.tile([128, B], f32, tag="outsb")
            nc.vector.tensor_mul(out_sb[:], mix_sb[:], rec_sb[:])
            # un-permute rows on the way out: sbuf partition 32q+v -> row 4v+q
            # (one DMA per strip: multi-dim partition APs don't lower in DMA)
            out_v = out_d.rearrange("(v q) j -> v q j", q=NQ)
            for q in range(NQ):
                eng = nc.sync if q % 2 == 0 else nc.gpsimd
                eng.dma_start(out_v[:, q, :], out_sb[32 * q:32 * q + NV, :])

    nc.compile()
    return nc


def _prep_inputs(img_emb, txt_emb, W1a, W1b, b1, W2, b2,
                 Wt1a, Wt1b, bt1, Wt2, bt2):
    """Host-side layout prep: transposes + delta-expanded reduction weights."""
    import ml_dtypes
    bf = ml_dtypes.bfloat16
    f = np.float32
    txtT = np.ascontiguousarray(np.asarray(txt_emb, f).T).astype(bf)
    w1aT = np.ascontiguousarray(np.asarray(W1a, f).T).astype(bf)
    w1bT = np.ascontiguousarray(np.asarray(W1b, f).T).astype(bf)
    wt1aT = np.ascontiguousarray(np.asarray(Wt1a, f).T).astype(bf)
    wt1bT = np.ascontiguousarray(np.asarray(Wt1b, f).T).astype(bf)
    b1c = np.ascontiguousarray(np.asarray(b1, f).reshape(DC_SIM, 128).T).astype(bf)
    bt1c = np.ascontiguousarray(np.asarray(bt1, f).reshape(DC_TMP, 128).T).astype(bf)

    def delta_expand(w, n_dc):
        w = np.asarray(w, f).reshape(n_dc, 128)
        s = np.zeros((n_dc, 128, NV, 32), f)
        for v in range(NV):
            s[:, :, v, v] = w
        return s.reshape(n_dc, 128, 32 * NV).astype(bf)

    s_sim = delta_expand(W2, DC_SIM)
    s_tmp = delta_expand(Wt2, DC_TMP)
    bias2 = np.empty((128, 2), f)
    bias2[:, 0] = f(b2)
    bias2[:, 1] = f(bt2)

    img = np.asarray(img_emb, f)
    shared = dict(txtT=txtT, w1aT=w1aT, w1bT=w1bT, wt1aT=wt1aT, wt1bT=wt1bT,
                  b1c=b1c, bt1c=bt1c, s_sim=s_sim, s_tmp=s_tmp, bias2=bias2)
    in_maps = []
    for m in range(NCORES):
        im = dict(shared)
        rows = img[m * ROWS:(m + 1) * ROWS, :]          # [64, 512]
        im["imgT"] = np.ascontiguousarray(rows.T).astype(bf)
        # permuted strip layout: column 32q+v = row 4v+q, v<NV; rest zero
        p = np.zeros((D, 128), f)
        for i in range(ROWS):
            q, v = i % NQ, i // NQ
            p[:, 32 * q + v] = (7.0 / 3.0) * rows[i, :]
        im["imgTp"] = p.astype(bf)
        in_maps.append(im)
    return in_maps


def _run(inputs, trace=False, tmpdir=None):
    from concourse.bass_utils import run_bass_kernel_spmd

    if "nc" not in _CACHE:
        _CACHE["nc"] = _build()
    nc = _CACHE["nc"]
    in_maps = _prep_inputs(**inputs)
    res = run_bass_kernel_spmd(nc, in_maps, core_ids=list(range(NCORES)),
                               trace=trace, tmpdir=tmpdir)
    out = np.concatenate([res.results[m]["out"] for m in range(NCORES)], axis=0)
    return out.astype(np.float32), res


def kernel(**inputs):
    out, _ = _run(inputs)
    return out


# revision 23
# speedup vs baseline: 1.0370x; 1.0052x over previous
"""Trainium2 Bass kernel for AdaptiveSimilarityLearning (pairwise MLP gate).

Computes, for B=512 image/text embeddings (D=512):
  img_h = img @ W1a.T ; txt_h = txt @ W1b.T
  adaptive_sim = sigmoid(sum_d relu(img_h[i,d]+txt_h[j,d]+b1[d]) * W2[d] + b2)
  adaptive_temp = 0.01 + 0.2*sigmoid(sum_d relu(img_t+txt_t+bt1)*Wt2 + bt2)
  out = (0.7 * img@txt.T + 0.3*adaptive_sim) / adaptive_temp

Sharding: rows of the B^2 grid split over 8 NeuronCores (64 rows each);
txt-side tensors and weights replicated. No collectives.

Per-core mapping:
  - d lives on SBUF partitions: txt_hT [d, j] tiles of [128, 512] (bf16).
  - relu(txt_hT[d,j] + img_hT[d,i]) is a per-partition-scalar op: DVE
    tensor_scalar(add, max 0) or ACT activation(Relu, bias) per (i, d-chunk).
  - The weighted partition-reduction sum_d h[d,j]*W2[d] runs on the PE as a
    bf16 matmul with a "delta" stationary S[k, 32v+m] = W2c[k]*(m==v).
  - Row i maps to PE column strip q=i%4 (tile_position=(0,32q)) at delta
    position v=i//4, so 4 consecutive rows' matmuls run on 4 independent
    32-column sub-array strips CONCURRENTLY (~54ns/MM effective) and the
    per-MM LDWEIGHTS is hidden. PSUM accumulator [128, 512] holds output
    row i at partition 32*(i%4) + i//4; cosine is computed in the same
    permuted layout via a host-permuted imgT, and the output DMA
    un-permutes rows on the way to DRAM.

All matmul operands are bf16 (fp32 matmuls lower to 2 half-rate passes on
TRN2); PSUM accumulation and the epilogue stay f32.
"""

import numpy as np

B = 512
D = 512
DH = 256
NCORES = 8
ROWS = B // NCORES  # 64 rows of the pairwise grid per core
KC = D // 128       # 4 contraction chunks
DC_SIM = D // 128   # 4 d-chunks (sim path)
DC_TMP = DH // 128  # 2 d-chunks (temp path)
NQ = 4              # PE column strips
NV = ROWS // NQ     # delta positions used per strip (16)

# measured per-[128,512]-tile costs (ns) used for static load balancing
_COST_DVE = 340.0
_COST_ACT = 704.0

_CACHE = {}


def _build():
    import concourse.tile as tile
    from concourse import bacc, mybir

    f32 = mybir.dt.float32
    bf16 = mybir.dt.bfloat16
    Alu = mybir.AluOpType
    Act = mybir.ActivationFunctionType

    nc = bacc.Bacc("TRN2", target_bir_lowering=False, debug=False,
                   num_devices=NCORES)

    dp = lambda name, shape: nc.dram_tensor(name, shape, bf16,
                                            kind="ExternalInput").ap()
    imgT = dp("imgT", [D, ROWS])        # k x i (natural row order)
    imgTp = dp("imgTp", [D, 128])       # k x permuted+padded strip layout
    txtT = dp("txtT", [D, B])
    w1aT = dp("w1aT", [D, D])
    w1bT = dp("w1bT", [D, D])
    wt1aT = dp("wt1aT", [D, DH])
    wt1bT = dp("wt1bT", [D, DH])
    b1c = dp("b1c", [128, DC_SIM])
    bt1c = dp("bt1c", [128, DC_TMP])
    s_sim = dp("s_sim", [DC_SIM, 128, 32 * NV])
    s_tmp = dp("s_tmp", [DC_TMP, 128, 32 * NV])
    bias2 = nc.dram_tensor("bias2", [128, 2], f32, kind="ExternalInput").ap()
    out_d = nc.dram_tensor("out", [ROWS, B], f32, kind="ExternalOutput").ap()

    with tile.TileContext(nc) as tc:
        with (
            tc.tile_pool(name="consts", bufs=1) as cp,
            tc.tile_pool(name="hpool", bufs=24) as hp,
            tc.tile_pool(name="psacc", bufs=1, space="PSUM") as pacc,
            tc.tile_pool(name="pspre", bufs=4, space="PSUM") as ppre,
            tc.tile_pool(name="epi", bufs=1) as ep,
        ):
            # ---- DMA inputs to SBUF -------------------------------------
            dma_rr = [0]

            def dma_in(dst, srcap):
                eng = nc.sync if dma_rr[0] % 2 == 0 else nc.gpsimd
                dma_rr[0] += 1
                eng.dma_start(dst, srcap)

            def load_chunks(ap, n_k, width, tag, dt=bf16):
                ts = []
                for k in range(n_k):
                    t = cp.tile([128, width], dt, tag=f"{tag}{k}",
                                name=f"{tag}{k}")
                    dma_in(t[:], ap[k * 128:(k + 1) * 128, :])
                    ts.append(t)
                return ts

            # temp-path tensors first so the temp projections start ASAP
            wt1bT_sb = load_chunks(wt1bT, KC, DH, "wt1bT")
            txtT_sb = load_chunks(txtT, KC, B, "txtT")
            bt1_sb = cp.tile([128, DC_TMP], bf16, tag="bt1")
            dma_in(bt1_sb[:], bt1c[:, :])
            wt1aT_sb = load_chunks(wt1aT, KC, DH, "wt1aT")
            imgT_sb = load_chunks(imgT, KC, ROWS, "imgT")
            s_tmp_sb = []
            for c in range(DC_TMP):
                t = cp.tile([128, 32 * NV], bf16, tag=f"stmp{c}", name=f"stmp{c}")
                dma_in(t[:], s_tmp[c, :, :])
                s_tmp_sb.append(t)
            w1bT_sb = load_chunks(w1bT, KC, D, "w1bT")
            b1_sb = cp.tile([128, DC_SIM], bf16, tag="b1")
            dma_in(b1_sb[:], b1c[:, :])
            w1aT_sb = load_chunks(w1aT, KC, D, "w1aT")
            s_sim_sb = []
            for c in range(DC_SIM):
                t = cp.tile([128, 32 * NV], bf16, tag=f"ssim{c}", name=f"ssim{c}")
                dma_in(t[:], s_sim[c, :, :])
                s_sim_sb.append(t)
            imgTp_sb = load_chunks(imgTp, KC, 128, "imgTp")
            bias2_sb = cp.tile([128, 2], f32, tag="bias2")
            dma_in(bias2_sb[:], bias2[:, :])

            # warm the ACT sigmoid table set during the DMA head
            warm = cp.tile([1, 1], f32, tag="warm")
            nc.vector.memset(warm[:], 0.0)
            nc.scalar.activation(warm[:], warm[:], Act.Sigmoid)

            # ---- precompute txt_hT/img_hT (and temp-net variants) -------
            def proj(lhsT_sb, rhs_sb, n_dc, rhs_w, bias_sb, tag, odt=bf16):
                """out[d, :] = sum_k lhsT[k, d] * rhs[k, :] (+ bias[d])."""
                outs = []
                for dc in range(n_dc):
                    ps = ppre.tile([128, rhs_w], f32, tag="pre", name="pre")
                    for k in range(KC):
                        nc.tensor.matmul(
                            ps[:], lhsT=lhsT_sb[k][:, dc * 128:(dc + 1) * 128],
                            rhs=rhs_sb[k][:], start=(k == 0), stop=(k == KC - 1))
                    o = cp.tile([128, rhs_w], odt, tag=f"{tag}{dc}",
                                name=f"{tag}{dc}")
                    if bias_sb is not None:
                        nc.scalar.activation(o[:], ps[:], Act.Identity,
                                             bias=bias_sb[:, dc:dc + 1])
                    else:
                        nc.scalar.copy(o[:], ps[:])
                    outs.append(o)
                return outs

            txt_tT = proj(wt1bT_sb, txtT_sb, DC_TMP, B, bt1_sb, "txttT")
            img_tT = proj(wt1aT_sb, imgT_sb, DC_TMP, ROWS, None, "imgtT", odt=f32)
            txt_hT = proj(w1bT_sb, txtT_sb, DC_SIM, B, b1_sb, "txthT")
            img_hT = proj(w1aT_sb, imgT_sb, DC_SIM, ROWS, None, "imghT", odt=f32)

            # ---- main pairwise loops ------------------------------------
            # init with ancillary load: DVE epilogue ~2.8us, ACT precompute
            # copies + sigmoids ~10us
            t_eng = [2800.0, 10000.0]  # dve, act

            def relu_pair(h, src, col):
                if t_eng[0] + _COST_DVE <= t_eng[1] + _COST_ACT:
                    t_eng[0] += _COST_DVE
                    nc.vector.tensor_scalar(h[:], src[:], col, 0.0,
                                            op0=Alu.add, op1=Alu.max)
                else:
                    t_eng[1] += _COST_ACT
                    nc.scalar.activation(h[:], src[:], Act.Relu, bias=col)

            def pair_loop(n_dc, txt_sb, img_sb, s_sb, ps, order):
                for i in order:
                    q, v = i % NQ, i // NQ
                    for c in range(n_dc):
                        h = hp.tile([128, B], bf16, tag="h", name="h")
                        relu_pair(h, txt_sb[c], img_sb[c][:, i:i + 1])
                        nc.tensor.matmul(
                            ps[32 * q:32 * q + 32, :],
                            lhsT=s_sb[c][:, 32 * v:32 * v + 32],
                            rhs=h[:],
                            start=(v == 0 and c == 0),
                            stop=(v == NV - 1 and c == n_dc - 1),
                            tile_position=(0, 32 * q))

            tmp_ps = pacc.tile([128, B], f32, tag="ptmp")
            sim_ps = pacc.tile([128, B], f32, tag="psim")

            # temp path first; its epilogue is emitted a few iterations into
            # the sim loop so the sigmoid doesn't stall the ACT FIFO at the
            # temp->sim boundary waiting for the last temp matmuls.
            pair_loop(DC_TMP, txt_tT, img_tT, s_tmp_sb, tmp_ps, range(ROWS))
            pair_loop(DC_SIM, txt_hT, img_hT, s_sim_sb, sim_ps, range(8))

            tmp_sb = ep.tile([128, B], f32, tag="tmp")
            nc.scalar.activation(tmp_sb[:], tmp_ps[:], Act.Sigmoid,
                                 bias=bias2_sb[:, 1:2])
            # tmp := (0.2*tmp + 0.01)/0.3 ; rec := 1/tmp
            # (the 0.3 from 0.3*adaptive_sim is folded here; 0.7 on cos is
            # folded into the host-scaled imgTp = 7/3 * img rows)
            nc.vector.tensor_scalar(tmp_sb[:], tmp_sb[:], 2.0 / 3.0, 1.0 / 30.0,
                                    op0=Alu.mult, op1=Alu.add)
            rec_sb = ep.tile([128, B], f32, tag="rec")
            nc.vector.reciprocal_approx_fast(rec_sb[:], tmp_sb[:])

            pair_loop(DC_SIM, txt_hT, img_hT, s_sim_sb, sim_ps, range(8, ROWS))

            # ---- cosine in permuted strip layout (PE tail work) ---------
            # psum partition 32q+v = output row 4v+q (via host-permuted imgTp,
            # which also carries the 7/3 mix factor)
            cos_ps = pacc.tile([128, B], f32, tag="cos")
            for k in range(KC):
                nc.tensor.matmul(cos_ps[:], lhsT=imgTp_sb[k][:],
                                 rhs=txtT_sb[k][:],
                                 start=(k == 0), stop=(k == KC - 1))

            # ---- epilogue (all in permuted strip layout) ----------------
            sim_sb = ep.tile([128, B], f32, tag="sim")
            nc.scalar.activation(sim_sb[:], sim_ps[:], Act.Sigmoid,
                                 bias=bias2_sb[:, 0:1])
            # out = ((7/3)*cos + sim) * (0.3/temp)
            mix_sb = ep.tile([128, B], f32, tag="mix")
            nc.vector.tensor_add(mix_sb[:], cos_ps[:], sim_sb[:])
            out_sb = ep.tile([128, B], f32, tag="outsb")
            nc.vector.tensor_mul(out_sb[:], mix_sb[:], rec_sb[:])
            # un-permute rows on the way out: sbuf partition 32q+v -> row 4v+q
            # (one DMA per strip: multi-dim partition APs don't lower in DMA)
            out_v = out_d.rearrange("(v q) j -> v q j", q=NQ)
            for q in range(NQ):
                eng = nc.sync if q % 2 == 0 else nc.gpsimd
                eng.dma_start(out_v[:, q, :], out_sb[32 * q:32 * q + NV, :])

    nc.compile()
    return nc


def _prep_inputs(img_emb, txt_emb, W1a, W1b, b1, W2, b2,
                 Wt1a, Wt1b, bt1, Wt2, bt2):
    """Host-side layout prep: transposes + delta-expanded reduction weights."""
    import ml_dtypes
    bf = ml_dtypes.bfloat16
    f = np.float32
    txtT = np.ascontiguousarray(np.asarray(txt_emb, f).T).astype(bf)
    w1aT = np.ascontiguousarray(np.asarray(W1a, f).T).astype(bf)
    w1bT = np.ascontiguousarray(np.asarray(W1b, f).T).astype(bf)
    wt1aT = np.ascontiguousarray(np.asarray(Wt1a, f).T).astype(bf)
    wt1bT = np.ascontiguousarray(np.asarray(Wt1b, f).T).astype(bf)
    b1c = np.ascontiguousarray(np.asarray(b1, f).reshape(DC_SIM, 128).T).astype(bf)
    bt1c = np.ascontiguousarray(np.asarray(bt1, f).reshape(DC_TMP, 128).T).astype(bf)

    def delta_expand(w, n_dc):
        w = np.asarray(w, f).reshape(n_dc, 128)
        s = np.zeros((n_dc, 128, NV, 32), f)
        for v in range(NV):
            s[:, :, v, v] = w
        return s.reshape(n_dc, 128, 32 * NV).astype(bf)

    s_sim = delta_expand(W2, DC_SIM)
    s_tmp = delta_expand(Wt2, DC_TMP)
    bias2 = np.empty((128, 2), f)
    bias2[:, 0] = f(b2)
    bias2[:, 1] = f(bt2)

    img = np.asarray(img_emb, f)
    shared = dict(txtT=txtT, w1aT=w1aT, w1bT=w1bT, wt1aT=wt1aT, wt1bT=wt1bT,
                  b1c=b1c, bt1c=bt1c, s_sim=s_sim, s_tmp=s_tmp, bias2=bias2)
    in_maps = []
    for m in range(NCORES):
        im = dict(shared)
        rows = img[m * ROWS:(m + 1) * ROWS, :]          # [64, 512]
        im["imgT"] = np.ascontiguousarray(rows.T).astype(bf)
        # permuted strip layout: column 32q+v = row 4v+q, v<NV; rest zero
        p = np.zeros((D, 128), f)
        for i in range(ROWS):
            q, v = i % NQ, i // NQ
            p[:, 32 * q + v] = (7.0 / 3.0) * rows[i, :]
        im["imgTp"] = p.astype(bf)
        in_maps.append(im)
    return in_maps


def _run(inputs, trace=False, tmpdir=None):
    from concourse.bass_utils import run_bass_kernel_spmd

    if "nc" not in _CACHE:
        _CACHE["nc"] = _build()
    nc = _CACHE["nc"]
    in_maps = _prep_inputs(**inputs)
    res = run_bass_kernel_spmd(nc, in_maps, core_ids=list(range(NCORES)),
                               trace=trace, tmpdir=tmpdir)
    out = np.concatenate([res.results[m]["out"] for m in range(NCORES)], axis=0)
    return out.astype(np.float32), res


def kernel(**inputs):
    out, _ = _run(inputs)
    return out


# revision 25
# speedup vs baseline: 1.0384x; 1.0013x over previous
"""Trainium2 Bass kernel for AdaptiveSimilarityLearning (pairwise MLP gate).

Computes, for B=512 image/text embeddings (D=512):
  img_h = img @ W1a.T ; txt_h = txt @ W1b.T
  adaptive_sim = sigmoid(sum_d relu(img_h[i,d]+txt_h[j,d]+b1[d]) * W2[d] + b2)
  adaptive_temp = 0.01 + 0.2*sigmoid(sum_d relu(img_t+txt_t+bt1)*Wt2 + bt2)
  out = (0.7 * img@txt.T + 0.3*adaptive_sim) / adaptive_temp

Sharding: rows of the B^2 grid split over 8 NeuronCores (64 rows each);
txt-side tensors and weights replicated. No collectives.

Per-core mapping:
  - d lives on SBUF partitions: txt_hT [d, j] tiles of [128, 512] (bf16).
  - relu(txt_hT[d,j] + img_hT[d,i]) is a per-partition-scalar op: DVE
    tensor_scalar(add, max 0) or ACT activation(Relu, bias) per (i, d-chunk).
  - The weighted partition-reduction sum_d h[d,j]*W2[d] runs on the PE as a
    bf16 matmul with a "delta" stationary S[k, 32v+m] = W2c[k]*(m==v).
  - Row i maps to PE column strip q=i%4 (tile_position=(0,32q)) at delta
    position v=i//4, so 4 consecutive rows' matmuls run on 4 independent
    32-column sub-array strips CONCURRENTLY (~54ns/MM effective) and the
    per-MM LDWEIGHTS is hidden. PSUM accumulator [128, 512] holds output
    row i at partition 32*(i%4) + i//4; cosine is computed in the same
    permuted layout via a host-permuted imgT, and the output DMA
    un-permutes rows on the way to DRAM.

All matmul operands are bf16 (fp32 matmuls lower to 2 half-rate passes on
TRN2); PSUM accumulation and the epilogue stay f32.
"""

import numpy as np

B = 512
D = 512
DH = 256
NCORES = 8
ROWS = B // NCORES  # 64 rows of the pairwise grid per core
KC = D // 128       # 4 contraction chunks
DC_SIM = D // 128   # 4 d-chunks (sim path)
DC_TMP = DH // 128  # 2 d-chunks (temp path)
NQ = 4              # PE column strips
NV = ROWS // NQ     # delta positions used per strip (16)

# measured per-[128,512]-tile costs (ns) used for static load balancing
_COST_DVE = 340.0
_COST_ACT = 704.0

_CACHE = {}


def _build():
    import concourse.tile as tile
    from concourse import bacc, mybir

    f32 = mybir.dt.float32
    bf16 = mybir.dt.bfloat16
    Alu = mybir.AluOpType
    Act = mybir.ActivationFunctionType

    nc = bacc.Bacc("TRN2", target_bir_lowering=False, debug=False,
                   num_devices=NCORES)

    dp = lambda name, shape: nc.dram_tensor(name, shape, bf16,
                                            kind="ExternalInput").ap()
    imgT = dp("imgT", [D, ROWS])        # k x i (natural row order)
    imgTp = dp("imgTp", [D, 128])       # k x permuted+padded strip layout
    txtT = dp("txtT", [D, B])
    w1aT = dp("w1aT", [D, D])
    w1bT = dp("w1bT", [D, D])
    wt1aT = dp("wt1aT", [D, DH])
    wt1bT = dp("wt1bT", [D, DH])
    b1c = dp("b1c", [128, DC_SIM])
    bt1c = dp("bt1c", [128, DC_TMP])
    s_sim = dp("s_sim", [DC_SIM, 128, 32 * NV])
    s_tmp = dp("s_tmp", [DC_TMP, 128, 32 * NV])
    bias2 = nc.dram_tensor("bias2", [128, 2], f32, kind="ExternalInput").ap()
    out_d = nc.dram_tensor("out", [ROWS, B], f32, kind="ExternalOutput").ap()

    with tile.TileContext(nc) as tc:
        with (
            tc.tile_pool(name="consts", bufs=1) as cp,
            tc.tile_pool(name="hpool", bufs=24) as hp,
            tc.tile_pool(name="psacc", bufs=1, space="PSUM") as pacc,
            tc.tile_pool(name="pspre", bufs=4, space="PSUM") as ppre,
            tc.tile_pool(name="epi", bufs=1) as ep,
        ):
            # ---- DMA inputs to SBUF -------------------------------------
            dma_rr = [0]

            def dma_in(dst, srcap):
                n = dma_rr[0]
                dma_rr[0] += 1
                if n < 12:
                    eng = [nc.sync, nc.gpsimd, nc.scalar][n % 3]
                else:
                    eng = nc.sync if n % 2 == 0 else nc.gpsimd
                eng.dma_start(dst, srcap)

            def load_chunks(ap, n_k, width, tag, dt=bf16):
                ts = []
                for k in range(n_k):
                    t = cp.tile([128, width], dt, tag=f"{tag}{k}",
                                name=f"{tag}{k}")
                    dma_in(t[:], ap[k * 128:(k + 1) * 128, :])
                    ts.append(t)
                return ts

            # temp-path tensors first so the temp projections start ASAP
            wt1bT_sb = load_chunks(wt1bT, KC, DH, "wt1bT")
            txtT_sb = load_chunks(txtT, KC, B, "txtT")
            bt1_sb = cp.tile([128, DC_TMP], bf16, tag="bt1")
            dma_in(bt1_sb[:], bt1c[:, :])
            wt1aT_sb = load_chunks(wt1aT, KC, DH, "wt1aT")
            imgT_sb = load_chunks(imgT, KC, ROWS, "imgT")
            s_tmp_sb = []
            for c in range(DC_TMP):
                t = cp.tile([128, 32 * NV], bf16, tag=f"stmp{c}", name=f"stmp{c}")
                dma_in(t[:], s_tmp[c, :, :])
                s_tmp_sb.append(t)
            w1bT_sb = load_chunks(w1bT, KC, D, "w1bT")
            b1_sb = cp.tile([128, DC_SIM], bf16, tag="b1")
            dma_in(b1_sb[:], b1c[:, :])
            w1aT_sb = load_chunks(w1aT, KC, D, "w1aT")
            s_sim_sb = []
            for c in range(DC_SIM):
                t = cp.tile([128, 32 * NV], bf16, tag=f"ssim{c}", name=f"ssim{c}")
                dma_in(t[:], s_sim[c, :, :])
                s_sim_sb.append(t)
            imgTp_sb = load_chunks(imgTp, KC, 128, "imgTp")
            bias2_sb = cp.tile([128, 2], f32, tag="bias2")
            dma_in(bias2_sb[:], bias2[:, :])

            # warm the ACT sigmoid table set during the DMA head
            warm = cp.tile([1, 1], f32, tag="warm")
            nc.vector.memset(warm[:], 0.0)
            nc.scalar.activation(warm[:], warm[:], Act.Sigmoid)

            # ---- precompute txt_hT/img_hT (and temp-net variants) -------
            def proj(lhsT_sb, rhs_sb, n_dc, rhs_w, bias_sb, tag, odt=bf16):
                """out[d, :] = sum_k lhsT[k, d] * rhs[k, :] (+ bias[d])."""
                outs = []
                for dc in range(n_dc):
                    ps = ppre.tile([128, rhs_w], f32, tag="pre", name="pre")
                    for k in range(KC):
                        nc.tensor.matmul(
                            ps[:], lhsT=lhsT_sb[k][:, dc * 128:(dc + 1) * 128],
                            rhs=rhs_sb[k][:], start=(k == 0), stop=(k == KC - 1))
                    o = cp.tile([128, rhs_w], odt, tag=f"{tag}{dc}",
                                name=f"{tag}{dc}")
                    if bias_sb is not None:
                        nc.scalar.activation(o[:], ps[:], Act.Identity,
                                             bias=bias_sb[:, dc:dc + 1])
                    else:
                        nc.scalar.copy(o[:], ps[:])
                    outs.append(o)
                return outs

            txt_tT = proj(wt1bT_sb, txtT_sb, DC_TMP, B, bt1_sb, "txttT")
            img_tT = proj(wt1aT_sb, imgT_sb, DC_TMP, ROWS, None, "imgtT", odt=f32)
            txt_hT = proj(w1bT_sb, txtT_sb, DC_SIM, B, b1_sb, "txthT")
            img_hT = proj(w1aT_sb, imgT_sb, DC_SIM, ROWS, None, "imghT", odt=f32)

            # ---- main pairwise loops ------------------------------------
            # init with ancillary load: DVE epilogue ~2.8us, ACT precompute
            # copies + sigmoids ~10us
            t_eng = [2800.0, 10000.0]  # dve, act

            def relu_pair(h, src, col):
                if t_eng[0] + _COST_DVE <= t_eng[1] + _COST_ACT:
                    t_eng[0] += _COST_DVE
                    nc.vector.tensor_scalar(h[:], src[:], col, 0.0,
                                            op0=Alu.add, op1=Alu.max)
                else:
                    t_eng[1] += _COST_ACT
                    nc.scalar.activation(h[:], src[:], Act.Relu, bias=col)

            def pair_loop(n_dc, txt_sb, img_sb, s_sb, ps, order):
                for i in order:
                    q, v = i % NQ, i // NQ
                    for c in range(n_dc):
                        h = hp.tile([128, B], bf16, tag="h", name="h")
                        relu_pair(h, txt_sb[c], img_sb[c][:, i:i + 1])
                        nc.tensor.matmul(
                            ps[32 * q:32 * q + 32, :],
                            lhsT=s_sb[c][:, 32 * v:32 * v + 32],
                            rhs=h[:],
                            start=(v == 0 and c == 0),
                            stop=(v == NV - 1 and c == n_dc - 1),
                            tile_position=(0, 32 * q))

            tmp_ps = pacc.tile([128, B], f32, tag="ptmp")
            sim_ps = pacc.tile([128, B], f32, tag="psim")

            # temp path first; its epilogue is emitted a few iterations into
            # the sim loop so the sigmoid doesn't stall the ACT FIFO at the
            # temp->sim boundary waiting for the last temp matmuls.
            pair_loop(DC_TMP, txt_tT, img_tT, s_tmp_sb, tmp_ps, range(ROWS))
            pair_loop(DC_SIM, txt_hT, img_hT, s_sim_sb, sim_ps, range(8))

            tmp_sb = ep.tile([128, B], f32, tag="tmp")
            nc.scalar.activation(tmp_sb[:], tmp_ps[:], Act.Sigmoid,
                                 bias=bias2_sb[:, 1:2])
            # tmp := (0.2*tmp + 0.01)/0.3 ; rec := 1/tmp
            # (the 0.3 from 0.3*adaptive_sim is folded here; 0.7 on cos is
            # folded into the host-scaled imgTp = 7/3 * img rows)
            nc.vector.tensor_scalar(tmp_sb[:], tmp_sb[:], 2.0 / 3.0, 1.0 / 30.0,
                                    op0=Alu.mult, op1=Alu.add)
            rec_sb = ep.tile([128, B], f32, tag="rec")
            nc.vector.reciprocal_approx_fast(rec_sb[:], tmp_sb[:])

            pair_loop(DC_SIM, txt_hT, img_hT, s_sim_sb, sim_ps, range(8, ROWS))

            # ---- cosine in permuted strip layout (PE tail work) ---------
            # psum partition 32q+v = output row 4v+q (via host-permuted imgTp,
            # which also carries the 7/3 mix factor)
            cos_ps = pacc.tile([128, B], f32, tag="cos")
            for k in range(KC):
                nc.tensor.matmul(cos_ps[:], lhsT=imgTp_sb[k][:],
                                 rhs=txtT_sb[k][:],
                                 start=(k == 0), stop=(k == KC - 1))

            # ---- epilogue (all in permuted strip layout) ----------------
            sim_sb = ep.tile([128, B], f32, tag="sim")
            nc.scalar.activation(sim_sb[:], sim_ps[:], Act.Sigmoid,
                                 bias=bias2_sb[:, 0:1])
            # out = ((7/3)*cos + sim) * (0.3/temp)
            mix_sb = ep.tile([128, B], f32, tag="mix")
            nc.vector.tensor_add(mix_sb[:], cos_ps[:], sim_sb[:])
            out_sb = ep.tile([128, B], f32, tag="outsb")
            nc.vector.tensor_mul(out_sb[:], mix_sb[:], rec_sb[:])
            # un-permute rows on the way out: sbuf partition 32q+v -> row 4v+q
            # (one DMA per strip: multi-dim partition APs don't lower in DMA)
            out_v = out_d.rearrange("(v q) j -> v q j", q=NQ)
            for q in range(NQ):
                eng = nc.sync if q % 2 == 0 else nc.gpsimd
                eng.dma_start(out_v[:, q, :], out_sb[32 * q:32 * q + NV, :])

    nc.compile()
    return nc


def _prep_inputs(img_emb, txt_emb, W1a, W1b, b1, W2, b2,
                 Wt1a, Wt1b, bt1, Wt2, bt2):
    """Host-side layout prep: transposes + delta-expanded reduction weights."""
    import ml_dtypes
    bf = ml_dtypes.bfloat16
    f = np.float32
    txtT = np.ascontiguousarray(np.asarray(txt_emb, f).T).astype(bf)
    w1aT = np.ascontiguousarray(np.asarray(W1a, f).T).astype(bf)
    w1bT = np.ascontiguousarray(np.asarray(W1b, f).T).astype(bf)
    wt1aT = np.ascontiguousarray(np.asarray(Wt1a, f).T).astype(bf)
    wt1bT = np.ascontiguousarray(np.asarray(Wt1b, f).T).astype(bf)
    b1c = np.ascontiguousarray(np.asarray(b1, f).reshape(DC_SIM, 128).T).astype(bf)
    bt1c = np.ascontiguousarray(np.asarray(bt1, f).reshape(DC_TMP, 128).T).astype(bf)

    def delta_expand(w, n_dc):
        w = np.asarray(w, f).reshape(n_dc, 128)
        s = np.zeros((n_dc, 128, NV, 32), f)
        for v in range(NV):
            s[:, :, v, v] = w
        return s.reshape(n_dc, 128, 32 * NV).astype(bf)

    s_sim = delta_expand(W2, DC_SIM)
    s_tmp = delta_expand(Wt2, DC_TMP)
    bias2 = np.empty((128, 2), f)
    bias2[:, 0] = f(b2)
    bias2[:, 1] = f(bt2)

    img = np.asarray(img_emb, f)
    shared = dict(txtT=txtT, w1aT=w1aT, w1bT=w1bT, wt1aT=wt1aT, wt1bT=wt1bT,
                  b1c=b1c, bt1c=bt1c, s_sim=s_sim, s_tmp=s_tmp, bias2=bias2)
    in_maps = []
    for m in range(NCORES):
        im = dict(shared)
        rows = img[m * ROWS:(m + 1) * ROWS, :]          # [64, 512]
        im["imgT"] = np.ascontiguousarray(rows.T).astype(bf)
        # permuted strip layout: column 32q+v = row 4v+q, v<NV; rest zero
        p = np.zeros((D, 128), f)
        for i in range(ROWS):
            q, v = i % NQ, i // NQ
            p[:, 32 * q + v] = (7.0 / 3.0) * rows[i, :]
        im["imgTp"] = p.astype(bf)
        in_maps.append(im)
    return in_maps


def _run(inputs, trace=False, tmpdir=None):
    from concourse.bass_utils import run_bass_kernel_spmd

    if "nc" not in _CACHE:
        _CACHE["nc"] = _build()
    nc = _CACHE["nc"]
    in_maps = _prep_inputs(**inputs)
    res = run_bass_kernel_spmd(nc, in_maps, core_ids=list(range(NCORES)),
                               trace=trace, tmpdir=tmpdir)
    out = np.concatenate([res.results[m]["out"] for m in range(NCORES)], axis=0)
    return out.astype(np.float32), res


def kernel(**inputs):
    out, _ = _run(inputs)
    return out
